# revision 4
# baseline (speedup 1.0000x reference)
"""Trainium2 Bass kernel for nn_EnhancedLocalAttention.

Reference semantics (B=4, L=4096, C=1024, H=16, D=64, WIN=256, step=128):
  qkv = x @ W_qkv + b_qkv -> q,k,v [B,H,L,D]
  overlapping windows n: tokens [n*128, n*128+256)
  per (b,h,n): S = (Q_win^T K_win)/8  (D x D, contracted over the 256 window
  tokens), P = softmax(S, axis=-1), O = P @ V_win^T  (D x W)
  regroup: rows of reshape(O, [256, 64]) laid at tokens n*256..n*256+255,
  slice to L -> only windows 0..15 survive; then @ W_out + b_out.

Sharding: 8 cores = (4 batches) x (2 window-halves of 8 windows each).
Each core consumes 9 x 128-token chunks and produces 2048 output rows.
"""

import threading

import numpy as np

import concourse.bacc as bacc
import concourse.masks as masks
import concourse.mybir as mybir
import concourse.tile as tile
from concourse._compat import get_trn_type
from concourse.bass_utils import run_bass_kernel_spmd

F32 = mybir.dt.float32
F32R = mybir.dt.float32r
F16 = mybir.dt.float16
EXP = mybir.ActivationFunctionType.Exp

B, L, C = 4, 4096, 1024
H, D, WIN, STEP = 16, 64, 256, 128
NCHUNK = 9            # 128-token chunks per core
NWIN = 8              # windows per core
TOK = NCHUNK * 128    # 1152 input tokens per core
OUT_ROWS = NWIN * 256 # 2048 output rows per core


def r32(ap):
    return ap.bitcast(F32R)


def build_program():
    nc = bacc.Bacc(
        get_trn_type() or "TRN2",
        target_bir_lowering=False,
        debug=False,
        num_devices=8,
    )
    xs = nc.dram_tensor("xs", [TOK, C], F32, kind="ExternalInput")
    wqkv = nc.dram_tensor("wqkv", [C, 3 * C], F32, kind="ExternalInput")
    bqkv = nc.dram_tensor("bqkv", [3 * C], F32, kind="ExternalInput")
    wout = nc.dram_tensor("wout", [C, C], F32, kind="ExternalInput")
    bout = nc.dram_tensor("bout", [C], F32, kind="ExternalInput")
    out = nc.dram_tensor("out", [OUT_ROWS, C], F32, kind="ExternalOutput")

    from contextlib import ExitStack

    with tile.TileContext(nc) as tc, ExitStack() as ctx:
        pool = lambda name, bufs: ctx.enter_context(tc.tile_pool(name=name, bufs=bufs))
        wq_pool = pool("wq", 8)
        wo_pool = pool("wo", 8)
        const_pool = pool("const", 1)
        x_pool = pool("x", 2)
        xt_pool = pool("xt", 10)
        q_pool = pool("q", 3)
        k_pool = pool("k", 3)
        v_pool = pool("v", 2)
        vt_pool = pool("vt", 20)
        at_pool = pool("at", 3)
        st_pool = pool("st", 4)
        yt_pool = pool("yt", 9)
        o_pool = pool("o", 2)
        ps = ctx.enter_context(tc.tile_pool(name="ps", bufs=8, space="PSUM"))

        # --- constants / weights ---
        idf32 = const_pool.tile([128, 128], F32, tag="idf32", name="idf32")
        masks.make_identity(nc, idf32[:])
        idf16 = const_pool.tile([128, 128], F16, tag="idf16", name="idf16")
        masks.make_identity(nc, idf16[:])
        ones_f = const_pool.tile([1, 128], F32, tag="ones_f", name="ones_f")
        nc.vector.memset(ones_f[:], 1.0)
        ones = const_pool.tile([1, 128], F32R, tag="ones", name="ones")
        nc.vector.tensor_copy(ones[:], ones_f[:])
        bq_sb = const_pool.tile([1, 3 * C], F32R, tag="bq", name="bq_sb")
        nc.sync.dma_start(bq_sb[:], bqkv.ap().rearrange("(a f) -> a f", a=1).bitcast(F32R))
        bo_sb = const_pool.tile([1, C], F32R, tag="bo", name="bo_sb")
        nc.sync.dma_start(bo_sb[:], bout.ap().rearrange("(a f) -> a f", a=1).bitcast(F32R))

        wq_sb = []
        for cb in range(8):
            t = wq_pool.tile([128, 3 * C], F32R, tag="wq", name=f"wq{cb}")
            nc.sync.dma_start(t[:], wqkv.ap()[cb * 128 : (cb + 1) * 128, :].bitcast(F32R))
            wq_sb.append(t)
        wo_sb = []
        for cb in range(8):
            t = wo_pool.tile([128, C], F32R, tag="wo", name=f"wo{cb}")
            nc.sync.dma_start(t[:], wout.ap()[cb * 128 : (cb + 1) * 128, :].bitcast(F32R))
            wo_sb.append(t)

        q_sb = [None] * NCHUNK
        k_sb = [None] * NCHUNK
        vt_sb = [[None] * 8 for _ in range(NCHUNK)]

        def qkv_chunk(r):
            """QKV projection for token chunk r -> q_sb[r], k_sb[r] (fp16,
            token-major, Q pre-scaled by 1/8) and vt_sb[r] (fp16 V^T)."""
            x_t = x_pool.tile([128, C], F32, tag="x", name="x_t")
            nc.sync.dma_start(x_t[:], xs.ap()[r * 128 : (r + 1) * 128, :])
            xt = []
            for cb in range(8):
                tp = ps.tile([128, 128], F32, tag="ps", name="tp")
                nc.tensor.transpose(tp[:], x_t[:, cb * 128 : (cb + 1) * 128], idf32[:])
                xtt = xt_pool.tile([128, 128], F32R, tag="xt", name="xtt")
                nc.vector.tensor_copy(xtt[:], tp[:])
                xt.append(xtt)

            # Q,K: features 0..2047 of qkv
            pq = [ps.tile([128, 512], F32, tag="ps", name=f"pq{i}") for i in range(4)]
            for cb in range(8):
                for i in range(4):
                    nc.tensor.matmul(
                        pq[i][:],
                        xt[cb][:],
                        wq_sb[cb][:, i * 512 : (i + 1) * 512],
                        start=(cb == 0),
                        stop=False,
                    )
            for i in range(4):
                nc.tensor.matmul(
                    pq[i][:],
                    ones[:, :],
                    bq_sb[:, i * 512 : (i + 1) * 512],
                    start=False,
                    stop=True,
                )
            qt = q_pool.tile([128, C], F16, tag="q", name="qt")
            nc.vector.tensor_scalar_mul(qt[:, 0:512], pq[0][:], 0.125)
            nc.vector.tensor_scalar_mul(qt[:, 512:1024], pq[1][:], 0.125)
            q_sb[r] = qt
            kt = k_pool.tile([128, C], F16, tag="k", name="kt")
            nc.vector.tensor_copy(kt[:, 0:512], pq[2][:])
            nc.vector.tensor_copy(kt[:, 512:1024], pq[3][:])
            k_sb[r] = kt

            # V: features 2048..3071
            pv = [ps.tile([128, 512], F32, tag="ps", name=f"pv{i}") for i in range(2)]
            for cb in range(8):
                for i in range(2):
                    nc.tensor.matmul(
                        pv[i][:],
                        xt[cb][:],
                        wq_sb[cb][:, 2048 + i * 512 : 2048 + (i + 1) * 512],
                        start=(cb == 0),
                        stop=False,
                    )
            for i in range(2):
                nc.tensor.matmul(
                    pv[i][:],
                    ones[:, :],
                    bq_sb[:, 2048 + i * 512 : 2048 + (i + 1) * 512],
                    start=False,
                    stop=True,
                )
            v_t = v_pool.tile([128, C], F16, tag="v", name="v_t")
            nc.vector.tensor_copy(v_t[:, 0:512], pv[0][:])
            nc.vector.tensor_copy(v_t[:, 512:1024], pv[1][:])
            for fb in range(8):
                tpv = ps.tile([128, 128], F16, tag="ps", name="tpv")
                nc.tensor.transpose(tpv[:], v_t[:, fb * 128 : (fb + 1) * 128], idf16[:])
                vtt = vt_pool.tile([128, 128], F16, tag="vt", name="vtt")
                nc.vector.tensor_copy(vtt[:], tpv[:])
                vt_sb[r][fb] = vtt

        def window(r):
            """Attention + out-projection for window with local index r
            (chunks r, r+1), writing output rows r*256..r*256+255."""
            yt = []
            for hp in range(8):
                h0 = 2 * hp
                s = ps.tile([128, 64], F32, tag="ps", name="s")
                for h, po in ((h0, 0), (h0 + 1, 64)):
                    c0, c1 = h * 64, h * 64 + 64
                    nc.tensor.matmul(
                        s[po : po + 64, :],
                        q_sb[r][:, c0:c1],
                        k_sb[r][:, c0:c1],
                        start=True,
                        stop=False,
                    )
                    nc.tensor.matmul(
                        s[po : po + 64, :],
                        q_sb[r + 1][:, c0:c1],
                        k_sb[r + 1][:, c0:c1],
                        start=False,
                        stop=True,
                    )
                p_exp = at_pool.tile([128, 64], F16, tag="p_exp", name="p_exp")
                ssum = st_pool.tile([128, 1], F32, tag="ssum", name="ssum")
                nc.scalar.activation(p_exp[:], s[:], EXP, accum_out=ssum[:])
                rs = st_pool.tile([128, 1], F32, tag="rs", name="rs")
                nc.vector.reciprocal(rs[:], ssum[:])
                p_n = at_pool.tile([128, 64], F16, tag="p_n", name="p_n")
                nc.vector.tensor_scalar_mul(p_n[:], p_exp[:], rs[:])
                ptp = ps.tile([128, 64], F16, tag="ps", name="ptp")
                nc.tensor.transpose(ptp[0:64, :], p_n[0:64, :], idf16[0:64, 0:64])
                nc.tensor.transpose(
                    ptp[64:128, :], p_n[64:128, :], idf16[64:128, 64:128]
                )
                ptsb = at_pool.tile([128, 64], F16, tag="ptsb", name="ptsb")
                nc.vector.tensor_copy(ptsb[:], ptp[:])

                ypsum = ps.tile([128, 256], F32, tag="ps", name="ypsum")
                for h, po in ((h0, 0), (h0 + 1, 64)):
                    for wq in range(4):
                        vtt = vt_sb[r + wq // 2][h // 2]
                        nc.tensor.matmul(
                            ypsum[po : po + 64, wq * 64 : (wq + 1) * 64],
                            vtt[po : po + 64, (wq % 2) * 64 : (wq % 2) * 64 + 64],
                            ptsb[po : po + 64, :],
                            start=True,
                            stop=True,
                        )
                ytt = yt_pool.tile([128, 256], F32R, tag="yt", name="ytt")
                # Y^T[c, d*4+wq] = ypsum[c, wq*64+d]  (torch-unfold regroup)
                nc.vector.tensor_copy(
                    ytt[:].rearrange("p (b a) -> p a b", a=4),
                    ypsum[:].rearrange("p (a b) -> p a b", a=4),
                )
                yt.append(ytt)

            for th in range(2):
                po_m = [ps.tile([128, 512], F32, tag="ps", name=f"pom{i}") for i in range(2)]
                for cb in range(8):
                    for mi in range(2):
                        nc.tensor.matmul(
                            po_m[mi][:],
                            yt[cb][:, th * 128 : (th + 1) * 128],
                            wo_sb[cb][:, mi * 512 : (mi + 1) * 512],
                            start=(cb == 0),
                            stop=False,
                        )
                for mi in range(2):
                    nc.tensor.matmul(
                        po_m[mi][:],
                        ones[:, :],
                        bo_sb[:, mi * 512 : (mi + 1) * 512],
                        start=False,
                        stop=True,
                    )
                ot = o_pool.tile([128, C], F32, tag="o", name="ot")
                nc.vector.tensor_copy(ot[:, 0:512], po_m[0][:])
                nc.vector.tensor_copy(ot[:, 512:1024], po_m[1][:])
                row = r * 256 + th * 128
                nc.sync.dma_start(out.ap()[row : row + 128, :], ot[:])

        for r in range(NCHUNK):
            qkv_chunk(r)
            if r >= 1:
                window(r - 1)

    nc.compile()
    return nc


_CACHE = {}
_LOCK = threading.Lock()


def _get_program():
    with _LOCK:
        if "nc" not in _CACHE:
            _CACHE["nc"] = build_program()
        return _CACHE["nc"]


def kernel(x, W_qkv, b_qkv, W_out, b_out):
    x = np.asarray(x, dtype=np.float32)
    W_qkv = np.asarray(W_qkv, dtype=np.float32)
    b_qkv = np.asarray(b_qkv, dtype=np.float32)
    W_out = np.asarray(W_out, dtype=np.float32)
    b_out = np.asarray(b_out, dtype=np.float32)

    nc = _get_program()
    in_maps = []
    for cid in range(8):
        b, half = cid // 2, cid % 2
        t0 = half * NWIN * STEP
        in_maps.append(
            {
                "xs": np.ascontiguousarray(x[b, t0 : t0 + TOK, :]),
                "wqkv": W_qkv,
                "bqkv": b_qkv,
                "wout": W_out,
                "bout": b_out,
            }
        )
    res = run_bass_kernel_spmd(nc, in_maps, core_ids=list(range(8)))
    out_full = np.empty((B, L, C), dtype=np.float32)
    for cid in range(8):
        b, half = cid // 2, cid % 2
        out_full[b, half * OUT_ROWS : (half + 1) * OUT_ROWS, :] = res.results[cid][
            "out"
        ]
    return out_full
